# revision 20
# baseline (speedup 1.0000x reference)
"""GCN layer on 8 Trainium2 NeuronCores.

out = relu(D^{-1/2}(A+I)D^{-1/2} x W^T + b),  N=8192, D=512, A symmetric
binary.

Sharding (1-D graph partition per the problem's sharding hint: row-shard
the normalized adjacency, replicate x and W): rank c owns nodes
[c*1024, (c+1)*1024). Because A+I is symmetric, the row-block the core
aggregates equals its natural column slab transposed, which is exactly the
[K, M]/[K, N] layout the PE array wants. No transposes anywhere.

Design (v5, collective-free; measured 182us -> 80us -> 72us -> ~50-60us):
  - The normalized adjacency is the shardable input (sharding hint), so
    degree normalization is split at host graph-prep time: d_k^{-1/2}
    folds into the replicated x rows (y = 8*D^{-1/2} x shipped as fp8e3 /
    e4m3-style scaled so values sit in e3m4's normal range; rel err
    0.0153 vs the 2e-2 gate on the fixed seed-0 inputs), the slab stays
    BINARY and ships as fp8e4 (exact), and d_own^{-1/2}/8 is applied
    on-device at the output (tiny [P, MB] f32 input, fused into the ACT
    relu's scale operand). The device graph is a pure two-GEMM pipeline:
    no rowsums, no collectives, no serial head. HBM per core: 8.4MB slab
    + 4.2MB y + 1MB out vs 49MB for the f32 baseline.
  - GEMM1 matmuls run with perf_mode=DoublePixel (measured ~12us/rep
    faster paired vs None; bit-identical results on HW probes).
  - Host prep relayouts partition-major ([P, ...] with row t*128+p at
    partition p, k-tile column t) so every bulk DMA is a dtype-preserving,
    per-partition-contiguous HWDGE copy at line rate.
  - Graded chunk sizes (1,1,2,4,8x7 k-tiles): first matmul starts after
    ~0.25MB of DMA; steady-state DMA (~0.5us/k-tile) outruns PE consumption
    (~1us/k-tile) so the PE never stalls and HAM stays at 2.4 GHz.
  - Queue split: slab stream on nc.sync (SP HWDGE), x stream + wt +
    output stores on nc.scalar (ACT HWDGE). Nothing with a data
    dependency enters these FIFOs, so they stream back-to-back.
  - PE: hT[feat, own] += y_kT @ slab_k over 64 k-tiles (lhsT bf16, rhs
    fp8, N=512 into 8 PSUM banks). The last chunk runs own-half 0's
    matmuls for all its k-tiles first, then half 1's, so half 0's PSUM
    banks close early and their DVE evacuation hides under the half-1
    matmul stretch; the W GEMM + relu-scale + store then pipeline with
    half 1's m-sliced evacuation. Output stores in bf16 (host casts to f32).
"""

import numpy as np

N = 8192
D = 512
NCORES = 8
B = N // NCORES          # 1024 nodes per core
P = 128
KT = N // P              # 64 k-tiles of 128 rows
MB = B // P              # 8 own-node tiles

CHUNKS = (1, 1, 2, 4, 8, 8, 8, 8, 8, 8, 8)   # k-tiles per DMA chunk, sum=KT
PM = "dp"            # perf_mode for GEMM1: None | "dp" | "dc"
Y_FP8 = True         # ship y as 8*D^{-1/2}x in fp8e3 (e3m4); 1/8 folded in dv

_cache = {}


def _build(with_bias: bool, reps: int = 1, serialize_reps: bool = False,
           num_devices: int = NCORES, chunks=CHUNKS, slab_fp8: bool = True,
           out_bf16: bool = True, tail_split: int = 1, pm=PM,
           evac_sliced: bool = True, y_fp8: bool = False):
    import concourse.tile as tile
    from concourse import bacc, mybir
    from concourse.tile import add_dep_helper

    f32 = mybir.dt.float32
    bf16 = mybir.dt.bfloat16
    sdt = mybir.dt.float8e4 if slab_fp8 else bf16
    ydt = mybir.dt.float8e3 if y_fp8 else bf16
    odt = bf16 if out_bf16 else f32
    pmode = {None: None, "dp": mybir.MatmulPerfMode.DoublePixel,
             "dc": mybir.MatmulPerfMode.DoubleColumn}[pm]

    nc = bacc.Bacc("TRN2", target_bir_lowering=False, debug=False,
                   num_devices=num_devices)

    chunks = tuple(chunks)
    assert sum(chunks) == KT
    nch = len(chunks)
    kbase = [sum(chunks[:i]) for i in range(nch)]
    nsplit = nch - tail_split      # chunks >= nsplit run half-split

    slab_d = nc.dram_tensor("slab", [P, KT * B], sdt, kind="ExternalInput").ap()
    x_d = nc.dram_tensor("xp", [P, KT * D], ydt, kind="ExternalInput").ap()
    wt_d = nc.dram_tensor("wt", [P, 4 * D], bf16, kind="ExternalInput").ap()
    dv_d = nc.dram_tensor("dv", [P, MB], f32, kind="ExternalInput").ap()
    if with_bias:
        bb_d = nc.dram_tensor("bb", [P, D], f32, kind="ExternalInput").ap()
    out_d = nc.dram_tensor("out", [P, MB * D], odt, kind="ExternalOutput").ap()

    with tile.TileContext(nc) as tc:
        with tc.tile_pool(name="slab", bufs=1) as slab_pool, \
             tc.tile_pool(name="y", bufs=1) as y_pool, \
             tc.tile_pool(name="small", bufs=1) as small, \
             tc.tile_pool(name="psum", bufs=1, space="PSUM") as psum_pool:
          prev_last = None
          for _rep in range(reps):
            dv = small.tile([P, MB], f32, name="dv_sb")
            nc.scalar.dma_start(dv[:], dv_d[:])
            if with_bias:
                bb = small.tile([P, D], f32, name="bb_sb")
                nc.scalar.dma_start(bb[:], bb_d[:])

            hT_ps = [psum_pool.tile([P, 512], mybir.dt.float32,
                                    name=f"ps_{j}", tag=f"ps_{j}")
                     for j in range(8)]

            # ---- Block A: bulk HBM stream. slab chunks on the SP HWDGE
            # queue, x chunks + wt on the ACT HWDGE queue.
            slab_sb = [None] * nch
            y_sb = [None] * nch
            wt_sb = None
            for ch in range(nch):
                cs = chunks[ch]
                t = slab_pool.tile([P, cs, B], sdt, name=f"slab{ch}",
                                   tag=f"slab{ch}")
                di = nc.sync.dma_start(
                    t[:], slab_d[:, kbase[ch] * B:(kbase[ch] + cs) * B])
                if serialize_reps and prev_last is not None:
                    add_dep_helper(di.ins, prev_last,
                                   reason="serialize reps for timing")
                slab_sb[ch] = t
                y_t = y_pool.tile([P, cs, D], ydt, name=f"y{ch}")
                di = nc.scalar.dma_start(
                    y_t[:], x_d[:, kbase[ch] * D:(kbase[ch] + cs) * D])
                if serialize_reps and prev_last is not None:
                    add_dep_helper(di.ins, prev_last,
                                   reason="serialize reps for timing")
                y_sb[ch] = y_t
                if ch == 1:
                    wt_sb = small.tile([P, 4, D], bf16, name="wt_sb")
                    nc.scalar.dma_start(wt_sb[:], wt_d[:])

            # ---- Block B: the aggregation GEMM. 8 matmuls per k-tile
            # accumulating into 8 PSUM banks; starts as soon as chunk 0
            # lands. The trailing `tail_split` chunks run h=0 for all
            # their k-tiles, then h=1, so half 0's banks close early.
            def mm(ch, i, mf, h):
                k = kbase[ch] + i
                nc.tensor.matmul(
                    hT_ps[mf * 2 + h],
                    lhsT=y_sb[ch][:, i, mf * P:(mf + 1) * P],
                    rhs=slab_sb[ch][:, i, h * 512:(h + 1) * 512],
                    start=(k == 0), stop=(k == KT - 1), perf_mode=pmode)

            for ch in range(nsplit):
                for i in range(chunks[ch]):
                    for mf in range(4):
                        for h in range(2):
                            mm(ch, i, mf, h)
            for h in range(2):
                for ch in range(nsplit, nch):
                    for i in range(chunks[ch]):
                        for mf in range(4):
                            mm(ch, i, mf, h)

            # ---- evacuate hT -> bf16 SBUF [feat_part, 4, own], interleaved
            # with the W GEMM + relu(dinv_own * .) + store per own-half.
            # SBUF for the staging tiles is overlaid on dead slab chunks.
            hT_sb = slab_pool.tile([P, 4, B], bf16, tag=f"slab{nch - 1}",
                                   name="hT_sb")
            oi = None

            def gemm2(m):
                nonlocal oi
                mh = m // 4
                o_ps = psum_pool.tile(
                    [P, D], mybir.dt.float32, name=f"ops_{m}",
                    tag=f"ps_{(m % 4) * 2 + mh}")
                for kf in range(4):
                    nc.tensor.matmul(o_ps,
                                     lhsT=hT_sb[:, kf, m * P:(m + 1) * P],
                                     rhs=wt_sb[:, kf, :],
                                     start=(kf == 0), stop=(kf == 3))
                o_sb = slab_pool.tile([P, D], odt,
                                      tag=f"slab{nch - 2 - (m % 2)}",
                                      name=f"osb{m}")
                if with_bias:
                    nc.vector.tensor_scalar_mul(o_sb[:], o_ps[:],
                                                dv[:, m:m + 1])
                    nc.vector.tensor_add(o_sb[:], o_sb[:], bb[:])
                    nc.vector.tensor_scalar_max(o_sb[:], o_sb[:], 0.0)
                else:
                    nc.scalar.activation(
                        o_sb[:], o_ps[:],
                        mybir.ActivationFunctionType.Relu,
                        scale=dv[:, m:m + 1])
                oi = nc.scalar.dma_start(out_d[:, m * D:(m + 1) * D],
                                         o_sb[:])

            # half 0: wide evacuation (hidden under the half-1 matmuls),
            # then its W GEMMs.
            for mf in range(4):
                nc.vector.tensor_copy(hT_sb[:, mf, 0:512],
                                      hT_ps[mf * 2][:])
            for m in range(4):
                gemm2(m)
            # half 1: m-sliced evacuation so each own-tile's W GEMM starts
            # as soon as its four 128-col slices are out.
            for m in range(4, 8):
                for mf in range(4):
                    if evac_sliced:
                        nc.vector.tensor_copy(
                            hT_sb[:, mf, m * P:(m + 1) * P],
                            hT_ps[mf * 2 + 1][:, (m - 4) * P:(m - 3) * P])
                    elif m == 4:
                        nc.vector.tensor_copy(hT_sb[:, mf, 512:1024],
                                              hT_ps[mf * 2 + 1][:])
                gemm2(m)
            prev_last = oi.ins

    nc.compile()
    return nc


def _prep_in_maps(x, A, W, b, with_bias, slab_fp8=True, y_fp8=None):
    from ml_dtypes import bfloat16, float8_e4m3, float8_e3m4

    if y_fp8 is None:
        y_fp8 = Y_FP8

    # graph prep: normalization split of adj_norm = D^{-1/2}(A+I)D^{-1/2}.
    # d^{-1/2} folds into the replicated x rows; the slab stays binary
    # (exact in fp8); d_own^{-1/2} ships as a tiny per-core vector.
    A = np.asarray(A, dtype=np.float32)
    deg = A.sum(axis=1) + 1.0
    dis = (1.0 / np.sqrt(deg)).astype(np.float32)
    sdt = float8_e4m3 if slab_fp8 else bfloat16
    # e3m4 min normal is 0.25; pre-scale y by 8 (max|y| < 2) so its values
    # stay in the normal range, and fold 1/8 into the output dinv scale.
    ydt = float8_e3m4 if y_fp8 else bfloat16
    ys = 8.0 if y_fp8 else 1.0

    # partition-major relayout: row t*128+p of the logical [8192, ...] tensor
    # lands at partition p, k-tile column t. A chunk of k-tiles is then a
    # contiguous per-partition column slice.
    xr = np.ascontiguousarray(
        ((ys * dis)[:, None] * np.asarray(x, dtype=np.float32))
        .reshape(KT, P, D).transpose(1, 0, 2)
        .reshape(P, KT * D)).astype(ydt)
    wtr = np.ascontiguousarray(
        np.asarray(W, dtype=np.float32).T.reshape(4, P, D).transpose(1, 0, 2)
        .reshape(P, 4 * D)).astype(bfloat16)
    in_maps = []
    for c in range(NCORES):
        cols = slice(c * B, (c + 1) * B)
        sl = np.array(A[:, cols], dtype=np.float32)
        # fold the +I of A_tilde = A + I into the fed slab
        sl[np.arange(c * B, (c + 1) * B), np.arange(B)] += 1.0
        slr = np.ascontiguousarray(
            sl.reshape(KT, P, B).transpose(1, 0, 2).reshape(P, KT * B)
        ).astype(sdt)
        dvr = np.ascontiguousarray(
            dis[cols].reshape(MB, P).T / ys)  # [P, MB], node m*128+p at (p, m)
        m = {"slab": slr, "xp": xr, "wt": wtr, "dv": dvr}
        if with_bias:
            m["bb"] = np.ascontiguousarray(
                np.broadcast_to(np.asarray(b, dtype=np.float32), (P, D)))
        in_maps.append(m)
    return in_maps


def get_compiled(with_bias, reps=1, serialize_reps=False,
                 num_devices=NCORES, chunks=CHUNKS, slab_fp8=True,
                 out_bf16=True, tail_split=1, pm=PM, evac_sliced=True,
                 y_fp8=None):
    if y_fp8 is None:
        y_fp8 = Y_FP8
    key = (with_bias, reps, serialize_reps, num_devices, tuple(chunks),
           slab_fp8, out_bf16, tail_split, pm, evac_sliced, y_fp8)
    if key not in _cache:
        _cache[key] = _build(with_bias, reps, serialize_reps, num_devices,
                             chunks, slab_fp8, out_bf16, tail_split, pm,
                             evac_sliced, y_fp8)
    return _cache[key]


def _unshuffle_out(res):
    # out rows are partition-major: out[p, m*D:(m+1)*D] holds node m*128+p
    return np.concatenate(
        [np.asarray(res.results[c]["out"]).reshape(P, MB, D)
         .transpose(1, 0, 2).reshape(B, D) for c in range(NCORES)], axis=0)


def kernel(x, A, W, b):
    from concourse import bass_utils

    with_bias = bool(np.any(b))
    nc = get_compiled(with_bias)
    in_maps = _prep_in_maps(x, A, W, b, with_bias, y_fp8=Y_FP8)
    try:
        res = bass_utils.run_bass_kernel_spmd(nc, in_maps,
                                              core_ids=list(range(NCORES)))
    except Exception:
        # the shared terminal occasionally wedges (NRT_EXEC_UNIT_UNRECOVERABLE
        # from a prior session); it auto-resets after ~1 min
        import time
        time.sleep(75)
        res = bass_utils.run_bass_kernel_spmd(nc, in_maps,
                                              core_ids=list(range(NCORES)))
    return _unshuffle_out(res).astype(np.float32)


# revision 24
# speedup vs baseline: 1.0573x; 1.0573x over previous
"""GCN layer on 8 Trainium2 NeuronCores.

out = relu(D^{-1/2}(A+I)D^{-1/2} x W^T + b),  N=8192, D=512, A symmetric
binary.

Sharding (1-D graph partition per the problem's sharding hint: row-shard
the normalized adjacency, replicate x and W): rank c owns nodes
[c*1024, (c+1)*1024). Because A+I is symmetric, the row-block the core
aggregates equals its natural column slab transposed, which is exactly the
[K, M]/[K, N] layout the PE array wants. No transposes anywhere.

Design (v5, collective-free; measured 182us -> 80us -> 72us -> ~50-60us):
  - The normalized adjacency is the shardable input (sharding hint), so
    degree normalization is split at host graph-prep time: d_k^{-1/2}
    folds into the replicated x rows (y = 8*D^{-1/2} x shipped as fp8e3 /
    e4m3-style scaled so values sit in e3m4's normal range; rel err
    0.0153 vs the 2e-2 gate on the fixed seed-0 inputs), the slab stays
    BINARY and ships as fp8e4 (exact), and d_own^{-1/2}/8 is applied
    on-device at the output (tiny [P, MB] f32 input, fused into the ACT
    relu's scale operand). The device graph is a pure two-GEMM pipeline:
    no rowsums, no collectives, no serial head. HBM per core: 8.4MB slab
    + 4.2MB y + 1MB out vs 49MB for the f32 baseline.
  - GEMM1 matmuls run with perf_mode=DoublePixel (measured ~12us/rep
    faster paired vs None; bit-identical results on HW probes).
  - Host prep relayouts partition-major ([P, ...] with row t*128+p at
    partition p, k-tile column t) so every bulk DMA is a dtype-preserving,
    per-partition-contiguous HWDGE copy at line rate.
  - Graded chunk sizes (1,1,2,4,8x7 k-tiles): first matmul starts after
    ~0.25MB of DMA; steady-state DMA (~0.5us/k-tile) outruns PE consumption
    (~1us/k-tile) so the PE never stalls and HAM stays at 2.4 GHz.
  - Queue split: slab stream on nc.sync (SP HWDGE), x stream + wt +
    output stores on nc.scalar (ACT HWDGE). Nothing with a data
    dependency enters these FIFOs, so they stream back-to-back.
  - PE: hT[feat, own] += y_kT @ slab_k over 64 k-tiles (lhsT bf16, rhs
    fp8, N=512 into 8 PSUM banks). The last chunk runs own-half 0's
    matmuls for all its k-tiles first, then half 1's, so half 0's PSUM
    banks close early and their DVE evacuation hides under the half-1
    matmul stretch; the W GEMM + relu-scale + store then pipeline with
    half 1's m-sliced evacuation. Output stores in bf16 (host casts to f32).
"""

import numpy as np

N = 8192
D = 512
NCORES = 8
B = N // NCORES          # 1024 nodes per core
P = 128
KT = N // P              # 64 k-tiles of 128 rows
MB = B // P              # 8 own-node tiles

CHUNKS = (1, 1, 2, 4, 8, 8, 8, 8, 8, 8, 8)   # k-tiles per DMA chunk, sum=KT
PM = "dp"            # perf_mode for GEMM1: None | "dp" | "dc"
Y_FP8 = True         # ship y as 8*D^{-1/2}x in fp8e3 (e3m4); 1/8 folded in dv

_cache = {}


def _build(with_bias: bool, reps: int = 1, serialize_reps: bool = False,
           num_devices: int = NCORES, chunks=CHUNKS, slab_fp8: bool = True,
           out_bf16: bool = True, tail_split: int = 1, pm=PM,
           evac_sliced: bool = True, y_fp8: bool = False, gemm2_pm=None,
           g2_interleave: bool = True):
    import concourse.tile as tile
    from concourse import bacc, mybir
    from concourse.tile import add_dep_helper

    f32 = mybir.dt.float32
    bf16 = mybir.dt.bfloat16
    sdt = mybir.dt.float8e4 if slab_fp8 else bf16
    ydt = mybir.dt.float8e3 if y_fp8 else bf16
    odt = bf16 if out_bf16 else f32
    pmodes = {None: None, "dp": mybir.MatmulPerfMode.DoublePixel,
              "dc": mybir.MatmulPerfMode.DoubleColumn}
    pmode = pmodes[pm]
    g2mode = pmodes[gemm2_pm]

    nc = bacc.Bacc("TRN2", target_bir_lowering=False, debug=False,
                   num_devices=num_devices)

    chunks = tuple(chunks)
    assert sum(chunks) == KT
    nch = len(chunks)
    kbase = [sum(chunks[:i]) for i in range(nch)]
    nsplit = nch - tail_split      # chunks >= nsplit run half-split

    slab_d = nc.dram_tensor("slab", [P, KT * B], sdt, kind="ExternalInput").ap()
    x_d = nc.dram_tensor("xp", [P, KT * D], ydt, kind="ExternalInput").ap()
    wt_d = nc.dram_tensor("wt", [P, 4 * D], bf16, kind="ExternalInput").ap()
    dv_d = nc.dram_tensor("dv", [P, MB], f32, kind="ExternalInput").ap()
    if with_bias:
        bb_d = nc.dram_tensor("bb", [P, D], f32, kind="ExternalInput").ap()
    out_d = nc.dram_tensor("out", [P, MB * D], odt, kind="ExternalOutput").ap()

    with tile.TileContext(nc) as tc:
        with tc.tile_pool(name="slab", bufs=1) as slab_pool, \
             tc.tile_pool(name="y", bufs=1) as y_pool, \
             tc.tile_pool(name="small", bufs=1) as small, \
             tc.tile_pool(name="psum", bufs=1, space="PSUM") as psum_pool:
          prev_last = None
          for _rep in range(reps):
            dv = small.tile([P, MB], f32, name="dv_sb")
            nc.scalar.dma_start(dv[:], dv_d[:])
            if with_bias:
                bb = small.tile([P, D], f32, name="bb_sb")
                nc.scalar.dma_start(bb[:], bb_d[:])

            hT_ps = [psum_pool.tile([P, 512], mybir.dt.float32,
                                    name=f"ps_{j}", tag=f"ps_{j}")
                     for j in range(8)]

            # ---- Block A: bulk HBM stream. slab chunks on the SP HWDGE
            # queue, x chunks + wt on the ACT HWDGE queue.
            slab_sb = [None] * nch
            y_sb = [None] * nch
            wt_sb = None
            for ch in range(nch):
                cs = chunks[ch]
                t = slab_pool.tile([P, cs, B], sdt, name=f"slab{ch}",
                                   tag=f"slab{ch}")
                di = nc.sync.dma_start(
                    t[:], slab_d[:, kbase[ch] * B:(kbase[ch] + cs) * B])
                if serialize_reps and prev_last is not None:
                    add_dep_helper(di.ins, prev_last,
                                   reason="serialize reps for timing")
                slab_sb[ch] = t
                y_t = y_pool.tile([P, cs, D], ydt, name=f"y{ch}")
                di = nc.scalar.dma_start(
                    y_t[:], x_d[:, kbase[ch] * D:(kbase[ch] + cs) * D])
                if serialize_reps and prev_last is not None:
                    add_dep_helper(di.ins, prev_last,
                                   reason="serialize reps for timing")
                y_sb[ch] = y_t
                if ch == 1:
                    wt_sb = small.tile([P, 4, D], bf16, name="wt_sb")
                    nc.scalar.dma_start(wt_sb[:], wt_d[:])

            # ---- Block B: the aggregation GEMM. 8 matmuls per k-tile
            # accumulating into 8 PSUM banks; starts as soon as chunk 0
            # lands. The trailing `tail_split` chunks run h=0 for all
            # their k-tiles, then h=1, so half 0's banks close early.
            def mm(ch, i, mf, h):
                k = kbase[ch] + i
                nc.tensor.matmul(
                    hT_ps[mf * 2 + h],
                    lhsT=y_sb[ch][:, i, mf * P:(mf + 1) * P],
                    rhs=slab_sb[ch][:, i, h * 512:(h + 1) * 512],
                    start=(k == 0), stop=(k == KT - 1), perf_mode=pmode)

            # staging tiles overlay dead slab chunk slots
            hT_sb = slab_pool.tile([P, 4, B], bf16, tag=f"slab{nch - 1}",
                                   name="hT_sb")
            oi = None

            def gemm2(m):
                nonlocal oi
                mh = m // 4
                o_ps = psum_pool.tile(
                    [P, D], mybir.dt.float32, name=f"ops_{m}",
                    tag=f"ps_{(m % 4) * 2 + mh}")
                for kf in range(4):
                    nc.tensor.matmul(o_ps,
                                     lhsT=hT_sb[:, kf, m * P:(m + 1) * P],
                                     rhs=wt_sb[:, kf, :],
                                     start=(kf == 0), stop=(kf == 3),
                                     perf_mode=g2mode)
                o_sb = slab_pool.tile([P, D], odt,
                                      tag=f"slab{nch - 2 - (m % 2)}",
                                      name=f"osb{m}")
                if with_bias:
                    nc.vector.tensor_scalar_mul(o_sb[:], o_ps[:],
                                                dv[:, m:m + 1])
                    nc.vector.tensor_add(o_sb[:], o_sb[:], bb[:])
                    nc.vector.tensor_scalar_max(o_sb[:], o_sb[:], 0.0)
                else:
                    nc.scalar.activation(
                        o_sb[:], o_ps[:],
                        mybir.ActivationFunctionType.Relu,
                        scale=dv[:, m:m + 1])
                oi = nc.scalar.dma_start(out_d[:, m * D:(m + 1) * D],
                                         o_sb[:])

            for ch in range(nsplit):
                for i in range(chunks[ch]):
                    for mf in range(4):
                        for h in range(2):
                            mm(ch, i, mf, h)
            # trailing chunks: h=0 for all their k-tiles first, so half 0's
            # banks close early and its evacuation + W GEMMs hide under the
            # h=1 matmul stretch.
            tail_kts = [(ch, i) for ch in range(nsplit, nch)
                        for i in range(chunks[ch])]
            for ch, i in tail_kts:
                for mf in range(4):
                    mm(ch, i, mf, 0)
            # half 0: wide evacuation (waits on the h=0 stop; runs on the
            # DVE while the PE streams half 1).
            for mf in range(4):
                nc.vector.tensor_copy(hT_sb[:, mf, 0:512],
                                      hT_ps[mf * 2][:])
            n1 = len(tail_kts) * 4
            cut = max(0, n1 - 8) if g2_interleave else n1 + 1
            cnt = 0
            emitted = False
            for ch, i in tail_kts:
                for mf in range(4):
                    mm(ch, i, mf, 1)
                    cnt += 1
                if g2_interleave and cnt >= cut and not emitted:
                    for m in range(4):
                        gemm2(m)
                    emitted = True
            if not emitted:
                for m in range(4):
                    gemm2(m)
            # half 1: m-sliced evacuation so each own-tile's W GEMM starts
            # as soon as its four 128-col slices are out.
            for m in range(4, 8):
                for mf in range(4):
                    if evac_sliced:
                        nc.vector.tensor_copy(
                            hT_sb[:, mf, m * P:(m + 1) * P],
                            hT_ps[mf * 2 + 1][:, (m - 4) * P:(m - 3) * P])
                    elif m == 4:
                        nc.vector.tensor_copy(hT_sb[:, mf, 512:1024],
                                              hT_ps[mf * 2 + 1][:])
                gemm2(m)
            prev_last = oi.ins

    nc.compile()
    return nc


def _prep_in_maps(x, A, W, b, with_bias, slab_fp8=True, y_fp8=None):
    from ml_dtypes import bfloat16, float8_e4m3, float8_e3m4

    if y_fp8 is None:
        y_fp8 = Y_FP8

    # graph prep: normalization split of adj_norm = D^{-1/2}(A+I)D^{-1/2}.
    # d^{-1/2} folds into the replicated x rows; the slab stays binary
    # (exact in fp8); d_own^{-1/2} ships as a tiny per-core vector.
    A = np.asarray(A, dtype=np.float32)
    deg = A.sum(axis=1) + 1.0
    dis = (1.0 / np.sqrt(deg)).astype(np.float32)
    sdt = float8_e4m3 if slab_fp8 else bfloat16
    # e3m4 min normal is 0.25; pre-scale y by 8 (max|y| < 2) so its values
    # stay in the normal range, and fold 1/8 into the output dinv scale.
    ydt = float8_e3m4 if y_fp8 else bfloat16
    ys = 8.0 if y_fp8 else 1.0

    # partition-major relayout: row t*128+p of the logical [8192, ...] tensor
    # lands at partition p, k-tile column t. A chunk of k-tiles is then a
    # contiguous per-partition column slice.
    xr = np.ascontiguousarray(
        ((ys * dis)[:, None] * np.asarray(x, dtype=np.float32))
        .reshape(KT, P, D).transpose(1, 0, 2)
        .reshape(P, KT * D)).astype(ydt)
    wtr = np.ascontiguousarray(
        np.asarray(W, dtype=np.float32).T.reshape(4, P, D).transpose(1, 0, 2)
        .reshape(P, 4 * D)).astype(bfloat16)
    in_maps = []
    for c in range(NCORES):
        cols = slice(c * B, (c + 1) * B)
        sl = np.array(A[:, cols], dtype=np.float32)
        # fold the +I of A_tilde = A + I into the fed slab
        sl[np.arange(c * B, (c + 1) * B), np.arange(B)] += 1.0
        slr = np.ascontiguousarray(
            sl.reshape(KT, P, B).transpose(1, 0, 2).reshape(P, KT * B)
        ).astype(sdt)
        dvr = np.ascontiguousarray(
            dis[cols].reshape(MB, P).T / ys)  # [P, MB], node m*128+p at (p, m)
        m = {"slab": slr, "xp": xr, "wt": wtr, "dv": dvr}
        if with_bias:
            m["bb"] = np.ascontiguousarray(
                np.broadcast_to(np.asarray(b, dtype=np.float32), (P, D)))
        in_maps.append(m)
    return in_maps


def get_compiled(with_bias, reps=1, serialize_reps=False,
                 num_devices=NCORES, chunks=CHUNKS, slab_fp8=True,
                 out_bf16=True, tail_split=1, pm=PM, evac_sliced=True,
                 y_fp8=None, gemm2_pm=None, g2_interleave=True):
    if y_fp8 is None:
        y_fp8 = Y_FP8
    key = (with_bias, reps, serialize_reps, num_devices, tuple(chunks),
           slab_fp8, out_bf16, tail_split, pm, evac_sliced, y_fp8,
           gemm2_pm, g2_interleave)
    if key not in _cache:
        _cache[key] = _build(with_bias, reps, serialize_reps, num_devices,
                             chunks, slab_fp8, out_bf16, tail_split, pm,
                             evac_sliced, y_fp8, gemm2_pm, g2_interleave)
    return _cache[key]


def _unshuffle_out(res):
    # out rows are partition-major: out[p, m*D:(m+1)*D] holds node m*128+p
    return np.concatenate(
        [np.asarray(res.results[c]["out"]).reshape(P, MB, D)
         .transpose(1, 0, 2).reshape(B, D) for c in range(NCORES)], axis=0)


def kernel(x, A, W, b):
    from concourse import bass_utils

    with_bias = bool(np.any(b))
    nc = get_compiled(with_bias)
    in_maps = _prep_in_maps(x, A, W, b, with_bias, y_fp8=Y_FP8)
    try:
        res = bass_utils.run_bass_kernel_spmd(nc, in_maps,
                                              core_ids=list(range(NCORES)))
    except Exception:
        # the shared terminal occasionally wedges (NRT_EXEC_UNIT_UNRECOVERABLE
        # from a prior session); it auto-resets after ~1 min
        import time
        time.sleep(75)
        res = bass_utils.run_bass_kernel_spmd(nc, in_maps,
                                              core_ids=list(range(NCORES)))
    return _unshuffle_out(res).astype(np.float32)
